# revision 1
# baseline (speedup 1.0000x reference)
"""Trainium2 Bass kernel v2 for nn_LIFcomplexLayer.

Sharding: 8 cores = 4 h-blocks x 2 b-halves. Core c owns h in
[128*(c%4), 128*(c%4)+128) and batches [16*(c//4), 16*(c//4)+16).
Each core computes its own Wx slice directly (no cross-core data exchange);
BN stats need only a pair AllReduce over {c, c+4}.

Phase A: stream x (16 batches), PE-transpose 128x128 tiles, fp32 matmuls
         into PSUM, ACT copies into the drive buffer dcol [P, T, 16] with
         per-tile sums; DVE computes sumsq.
Phase B: pair AllReduce of [P, 2] stats; BN factors gsc/hof [P,1]; bulk
         BN apply over dcol split across DVE/GpSimd/ACT.
Phase C: 3 DVE ops/step with a custom fused op
         LIF_STEP: out = ((Src0 > imm2) - Src0)*C0 + Src1*C1
           U_t:  ur_t = nar*negm(ur_{t-1}) + y_{t-1}
           W_t:  om_t = aisq*negm(ur_{t-1}) + ar*om_{t-1}
           Y_t:  y_t  = om_t + d_{t+1}        (stock TT)
         ur_t overwrites the consumed d-column in dcol.
Epilogue: chunked bulk spike threshold (GpSimd/ACT) + streaming DMA out.
"""

import sys

if "/opt/trn_rl_repo" not in sys.path:
    sys.path.insert(0, "/opt/trn_rl_repo")

import os
import numpy as np

B, T, I, H = 32, 2048, 512, 512
NCORES = 8
P = 128
HB = H // P            # 4 h-blocks
BLOC = 16              # batches per core
IC = I // P            # 4 i-chunks
TC = 4                 # t-chunks per batch in phase A
TCH = T // TC          # 512 t per chunk
NTOT = float(B * T)

TSTEPS = int(os.environ.get("LIF_TSTEPS", str(T)))

_CACHE = {}


def _register_lif_op():
    from concourse import dve_ops
    from concourse.dve_ops import DveOp
    from concourse.dve_spec import Spec, Src0, Src1, C0, C1, C2, lower
    from concourse.dve_uop import DveOpSpec

    name = "LIF_STEP_ANT"
    if name in dve_ops._SUB_OPCODE_FOR_NAME:
        return next(op for op in dve_ops.OPS if op.name == name)

    spec = Spec(
        body=((Src0 > C2) - Src0) * C0 + Src1 * C1,
        reference=lambda in0, in1, s0, s1, imm2: (
            ((in0 > imm2).astype(np.float32) - in0) * s0 + in1 * s1
        ),
    )
    row = max(dve_ops._SUB_OPCODE_FOR_NAME.values()) + 1
    assert row < 0x20
    shas = {}
    for ver in ("v3", "v4"):
        uops = lower(spec, ver=ver)
        s = DveOpSpec(name=name, opcode=row, uops=uops, rd1_en=True)
        shas[ver] = s.sha(ver)
    op = DveOp(name, spec, subdim=False, uops_sha=shas)
    dve_ops.OPS.append(op)
    dve_ops._SUB_OPCODE_FOR_NAME[name] = row
    dve_ops.CUSTOM_DVE_SPECS[name] = spec
    return op


def _build():
    import concourse.bass as bass
    import concourse.bacc as bacc
    import concourse.tile as tile
    from concourse import mybir
    from contextlib import ExitStack

    lif_op = _register_lif_op()

    dt = mybir.dt
    f32 = dt.float32
    Alu = mybir.AluOpType
    Act = mybir.ActivationFunctionType

    nc = bacc.Bacc(
        "TRN2", target_bir_lowering=False, debug=False, num_devices=NCORES
    )

    # per-core inputs
    x_d = nc.dram_tensor("x", [BLOC, T, I], f32, kind="ExternalInput").ap()
    wt_d = nc.dram_tensor("wt", [I, P], f32, kind="ExternalInput").ap()  # its h-block
    ident_d = nc.dram_tensor("ident", [P, P], f32, kind="ExternalInput").ap()
    sca_d = nc.dram_tensor("sca", [P, 8], f32, kind="ExternalInput").ap()
    init_d = nc.dram_tensor("init", [P, 2, BLOC], f32, kind="ExternalInput").ap()
    out_d = nc.dram_tensor("out", [P, T, BLOC], f32, kind="ExternalOutput").ap()

    with tile.TileContext(nc) as tc, ExitStack() as ctx:
        consts = ctx.enter_context(tc.tile_pool(name="consts", bufs=1))
        big = ctx.enter_context(tc.tile_pool(name="big", bufs=1))
        xin = ctx.enter_context(tc.tile_pool(name="xin", bufs=3))
        xtp = ctx.enter_context(tc.tile_pool(name="xtp", bufs=2))
        ppool = ctx.enter_context(tc.tile_pool(name="psumT", bufs=4, space="PSUM"))
        mpool = ctx.enter_context(tc.tile_pool(name="psumM", bufs=2, space="PSUM"))
        trash_p = ctx.enter_context(tc.tile_pool(name="trash", bufs=2))
        small = ctx.enter_context(tc.tile_pool(name="small", bufs=1))
        state_p = ctx.enter_context(tc.tile_pool(name="state", bufs=1))
        dram = ctx.enter_context(tc.tile_pool(name="dram", bufs=1, space="DRAM"))

        wt_sb = consts.tile([P, IC, P], f32)  # [i(128p), ic, h(128)]
        nc.sync.dma_start(wt_sb[:], wt_d.rearrange("(ic p) h -> p ic h", p=P))
        ident_sb = consts.tile([P, P], f32)
        nc.sync.dma_start(ident_sb[:], ident_d[:])
        sca = consts.tile([P, 8], f32)  # nar, aisq, ar, bg, bb
        nc.sync.dma_start(sca[:], sca_d[:])
        init_sb = consts.tile([P, 2, BLOC], f32)  # A0, W0
        nc.sync.dma_start(init_sb[:], init_d[:])

        nar = sca[:, 0:1]
        aisq = sca[:, 1:2]
        ar = sca[:, 2:3]
        bg = sca[:, 3:4]   # b*gamma
        bb = sca[:, 4:5]   # b*beta

        # drive/output buffer: [P(h), T, BLOC]; column t contiguous 16
        dcol = big.tile([P, T, BLOC], f32)
        sumS = small.tile([P, BLOC * TC], f32)
        sumQ = small.tile([P, BLOC * TC], f32)

        # ---- phase A ----
        for b in range(BLOC):
            for tcix in range(TC):
                xr = xin.tile([P, TC, I], f32)  # [t(128p), tt, i]
                nc.sync.dma_start(
                    xr[:],
                    x_d[b, tcix * TCH : (tcix + 1) * TCH, :].rearrange(
                        "(tt p) i -> p tt i", p=P
                    ),
                )
                xt = xtp.tile([P, IC, TCH], f32)  # [i(128p), ic, t]
                for tt in range(TC):
                    pt4 = ppool.tile([P, IC, P], f32)
                    for ic in range(IC):
                        nc.tensor.transpose(
                            pt4[:, ic, :], xr[:, tt, ic * P : (ic + 1) * P],
                            ident_sb[:],
                        )
                    cdst = xt[:, :, tt * P : (tt + 1) * P]
                    if (b * TC + tcix + tt) % 2 == 0:
                        nc.scalar.copy(cdst, pt4[:])
                    else:
                        nc.vector.tensor_scalar(cdst, pt4[:], 1.0, None, op0=Alu.mult)
                pm = mpool.tile([P, TCH], f32)
                for ic in range(IC):
                    nc.tensor.matmul(
                        pm[:],
                        lhsT=wt_sb[:, ic, :],
                        rhs=xt[:, ic, :],
                        start=(ic == 0),
                        stop=(ic == IC - 1),
                    )
                idx = b * TC + tcix
                dst = dcol[:, tcix * TCH : (tcix + 1) * TCH, b]
                nc.scalar.activation(
                    dst, pm[:], Act.Identity, accum_out=sumS[:, idx : idx + 1]
                )
                trash = trash_p.tile([P, TCH], f32)
                nc.vector.scalar_tensor_tensor(
                    trash[:],
                    dst,
                    1.0,
                    dst,
                    op0=Alu.bypass,
                    op1=Alu.mult,
                    accum_out=sumQ[:, idx : idx + 1],
                )

        # ---- phase B: pair stats all-reduce + BN factors ----
        stats = small.tile([P, 2], f32)
        nc.vector.tensor_reduce(
            stats[:, 0:1], sumS[:], axis=mybir.AxisListType.X, op=Alu.add
        )
        nc.vector.tensor_reduce(
            stats[:, 1:2], sumQ[:], axis=mybir.AxisListType.X, op=Alu.add
        )
        cc_in = dram.tile([P, 2], f32)
        cc_out = dram.tile([P, 2], f32)
        nc.sync.dma_start(cc_in[:], stats[:])
        nc.gpsimd.collective_compute(
            "AllReduce",
            Alu.add,
            replica_groups=[[0, 1], [2, 3], [4, 5], [6, 7]],
            ins=[cc_in.opt()],
            outs=[cc_out.opt()],
        )
        gstats = small.tile([P, 2], f32)
        nc.sync.dma_start(gstats[:], cc_out[:])

        mean = small.tile([P, 1], f32)
        tmp = small.tile([P, 1], f32)
        var = small.tile([P, 1], f32)
        inv = small.tile([P, 1], f32)
        gsc = small.tile([P, 1], f32)
        hof = small.tile([P, 1], f32)
        nc.vector.tensor_scalar(mean[:], gstats[:, 0:1], 1.0 / NTOT, None, op0=Alu.mult)
        nc.vector.tensor_scalar(tmp[:], gstats[:, 1:2], 1.0 / NTOT, None, op0=Alu.mult)
        nc.vector.tensor_tensor(var[:], mean[:], mean[:], op=Alu.mult)
        nc.vector.tensor_tensor(var[:], tmp[:], var[:], op=Alu.subtract)
        nc.vector.tensor_scalar(var[:], var[:], 1e-5, None, op0=Alu.add)
        nc.scalar.sqrt(tmp[:], var[:])
        nc.vector.reciprocal(inv[:], tmp[:])
        nc.vector.tensor_tensor(gsc[:], bg[:], inv[:], op=Alu.mult)
        nc.vector.tensor_tensor(tmp[:], mean[:], gsc[:], op=Alu.mult)
        nc.vector.tensor_tensor(hof[:], bb[:], tmp[:], op=Alu.subtract)

        # bulk BN apply on dcol: d = gsc*Wx + hof. All on Scalar — keeps the
        # DVE queue free so the recurrence starts as soon as chunk 0 lands
        # (the recurrence consumes raw d columns far slower than ACT BN's).
        NSPLIT = 16
        CH = T // NSPLIT
        for k in range(NSPLIT):
            dst = dcol[:, k * CH : (k + 1) * CH, :].rearrange("p t b -> p (t b)")
            nc.scalar.activation(dst, dst, Act.Identity, bias=hof[:], scale=gsc[:])

        # ---- phase C ----
        om = state_p.tile([P, 2, BLOC], f32)
        yy = state_p.tile([P, 2, BLOC], f32)

        # ur_0 = A0 + d_0 (in place); om_0 = W0; y_0 = om_0 + d_1
        nc.vector.tensor_tensor(dcol[:, 0, :], init_sb[:, 0, :], dcol[:, 0, :],
                                op=Alu.add)
        nc.vector.tensor_tensor(yy[:, 0, :], init_sb[:, 1, :], dcol[:, 1, :],
                                op=Alu.add)
        nc.scalar.copy(om[:, 0, :], init_sb[:, 1, :])

        # spike + DMA for a completed chunk of columns [lo, hi). Two ACT passes
        # (Sign(2x-1) -> {-1,0,1}, then Relu -> {0,1}) keep it entirely on the
        # Scalar engine: ACT has its own SBUF ports, so unlike GpSimd/DVE it
        # does not steal recurrence bandwidth.
        def flush_chunk(lo, hi):
            sl = dcol[:, lo:hi, :].rearrange("p t b -> p (t b)")
            nc.scalar.activation(sl, sl, Act.Sign, scale=sca[:, 6:7],
                                 bias=sca[:, 5:6])
            nc.scalar.activation(sl, sl, Act.Relu)
            nc.sync.dma_start(out_d[:, lo:hi, :], dcol[:, lo:hi, :])

        NOUT = 16
        OCH = T // NOUT

        for t in range(1, TSTEPS):
            pi = (t - 1) % 2
            ci = t % 2
            nc.vector._custom_dve(
                lif_op, out=dcol[:, t, :], in0=dcol[:, t - 1, :], in1=yy[:, pi, :],
                s0=nar, s1=1.0, imm2=0.5,
            )
            if t <= TSTEPS - 2:
                nc.vector._custom_dve(
                    lif_op, out=om[:, ci, :], in0=dcol[:, t - 1, :], in1=om[:, pi, :],
                    s0=aisq, s1=ar, imm2=0.5,
                )
                nc.vector.tensor_tensor(yy[:, ci, :], om[:, ci, :],
                                        dcol[:, t + 1, :], op=Alu.add)
            if TSTEPS == T and t % OCH == 0 and t >= OCH:
                flush_chunk(t - OCH, t)
            elif TSTEPS == T and t == T - 8:
                # flush most of the final chunk early so the post-loop tail
                # only covers the last 8 columns
                flush_chunk(T - OCH, T - 8)

        if TSTEPS == T:
            flush_chunk(T - 8, T)
        else:
            flush_chunk(0, T)

    nc.compile()
    return nc


def _prep_host(W, log_log_alpha, log_dt, alpha_img, b, gamma, beta):
    lla = np.exp(log_log_alpha.astype(np.float32))
    dtv = np.exp(log_dt.astype(np.float32)).astype(np.float32)
    z = (-lla.astype(np.complex64) + 1j * alpha_img.astype(np.complex64)) * dtv
    alpha = np.exp(z.astype(np.complex64))
    a_r = alpha.real.astype(np.float32)  # [H]
    a_i = alpha.imag.astype(np.float32)
    wt = np.ascontiguousarray(W.T.astype(np.float32))  # [I, H]
    ident = np.eye(P, dtype=np.float32)
    return wt, ident, a_r, a_i


def kernel(x, W, log_log_alpha, log_dt, alpha_img, b, gamma, beta,
           u0_real, u0_imag, s0):
    from concourse.bass_utils import run_bass_kernel_spmd

    if "nc" not in _CACHE:
        _CACHE["nc"] = _build()
    nc = _CACHE["nc"]

    wt, ident, a_r, a_i = _prep_host(
        W, log_log_alpha, log_dt, alpha_img, b, gamma, beta
    )

    in_maps = []
    for c in range(NCORES):
        # HBM-pair-local stats: cores (2j, 2j+1) share h-block j and differ
        # only in b-half, so the BN AllReduce pairs stay intra-HBM-domain.
        j = c // 2           # h-block
        k = c % 2            # b-half
        hs = slice(128 * j, 128 * j + 128)
        bs = slice(16 * k, 16 * k + 16)

        arh = a_r[hs][:, None]  # [P,1]
        aih = a_i[hs][:, None]

        sca = np.zeros((P, 8), np.float32)
        sca[:, 0] = -arh[:, 0]
        sca[:, 1] = (aih * aih)[:, 0]
        sca[:, 2] = arh[:, 0]
        sca[:, 3] = (b * gamma)[hs].astype(np.float32)
        sca[:, 4] = (b * beta)[hs].astype(np.float32)
        sca[:, 5] = -1.0
        sca[:, 6] = 2.0

        u0r = u0_real[bs][:, hs].astype(np.float32).T  # [P, 16]
        u0i = u0_imag[bs][:, hs].astype(np.float32).T
        s0h = s0[bs][:, hs].astype(np.float32).T
        m_init = u0r - s0h
        init = np.zeros((P, 2, BLOC), np.float32)
        init[:, 0] = arh * m_init - aih * u0i               # A0
        init[:, 1] = -aih * aih * m_init - aih * arh * u0i  # W0 = omega_0

        in_maps.append({
            "x": np.ascontiguousarray(x[bs].astype(np.float32)),
            "wt": np.ascontiguousarray(wt[:, hs]),
            "ident": ident,
            "sca": sca,
            "init": init,
        })

    res = run_bass_kernel_spmd(
        nc,
        in_maps,
        core_ids=list(range(NCORES)),
        trace=bool(int(os.environ.get("LIF_TRACE", "0"))),
    )
    _CACHE["last_res"] = res
    out = np.empty((B, T, H), np.float32)
    for c in range(NCORES):
        j = c // 2
        k = c % 2
        o = res.results[c]["out"]  # [P(h), T, BLOC]
        out[16 * k : 16 * k + 16, :, 128 * j : 128 * j + 128] = o.transpose(2, 1, 0)
    return out

